# revision 80
# baseline (speedup 1.0000x reference)
"""Multi-head causal attention (B=4, S=2048, D=1024, H=16) on 8 NeuronCores.

Sharding: batch x head-group. Core c handles batch b = c//2 and head group
g = c%2 (8 heads of 64 dims each). Wq/Wk/Wv are column-split per head group
(Megatron column-parallel), Wo is row-split; each core returns a partial
output [S, D] which the host sums over the two head-group cores per batch.

Device kernel (identical SPMD program on all 8 cores, bf16 compute,
fp32 accumulation):
  1. QT/KT = (X @ Wg)^T computed directly in transposed layout
     (dk on partitions) so attention matmuls need no on-device transpose.
     V computed in natural layout [sk, dv] and packed with a ones-column
     per head (denominator trick).
  2. Per head: L^T tiles [sk=128, sq=512] = KT_h^T-slices @ QT_h (K=64),
     exp via ScalarE with the 1/sqrt(dk) folded into the activation scale,
     causal masking on diagonal tiles via a preloaded 0/1 mask multiply,
     then C~^T[65, sq] += Vtilde^T @ A^T accumulated over sk chunks into
     two 512-wide PSUM halves: rows 0..63 are the unnormalized context^T,
     row 64 the softmax denominator. Each half is normalized (reciprocal +
     DMA row-broadcast + VectorE multiply) as soon as its accumulation
     stops, freeing PSUM early and unblocking the output projection at
     512-column granularity.
  3. Output projection: ct pair tiles [128(dv), 128(sq)] @ Wo rows,
     accumulating all 8 heads (K=128 per matmul), PSUM -> DRAM f32.

Schedule: a short prelude (V proj j0..7, K/Q proj for head-pair 0 over the
first half of the sequence) gets attention running ~15us in; all remaining
projection work (V j8..15, K/Q m1..3 + second halves, output projection)
is emitted as fine-grained filler between attention jk-steps so TensorE
stays busy while ScalarE (exp) paces the attention pipeline. DMAs are
split per-128-row chunk and per-1024-column half, ordered by first use.
"""
import json
from collections import deque

import numpy as np
import ml_dtypes

BF16 = ml_dtypes.bfloat16

B, S, D = 4, 2048, 1024
H = 16
DK = 64          # per-head dim
HPG = 8          # heads per group
GW = HPG * DK    # group width = 512
N_CORES = 8

_nc_cache = {}
TUNE = {"apool": 8, "xh": 36, "psL": 2, "psP": 2, "fill": 1,
        "js23_base": 48, "js23_step": 36, "opj0_base": 110, "opj0_step": 10,
        "pooldma": 0, "actcopy": 0, "warm_last": 0}


def _apply_compat_patches():
    """This container's walrus rejects instructions carrying more than one
    sem-wait ("Too many sync wait commands"). Split excess waits onto NoOps
    on the same engine, patched into every compile path."""
    import concourse.bass_utils as bass_utils

    if getattr(bass_utils, "_wait_split_patched", False):
        return
    _orig = bass_utils.compile_bir_kernel
    seq = [0]

    def split_bir_waits(bir, limit=1):
        for fn in bir.get("functions", []):
            for bb in fn.get("blocks", []):
                out, changed = [], False
                for ins in bb.get("instructions", []):
                    si = ins.get("sync_info")
                    ow = (si or {}).get("on_wait") or []
                    if len(ow) > limit:
                        changed = True
                        extra, keep = ow[:-limit], ow[-limit:]
                        for i in range(0, len(extra), limit):
                            seq[0] += 1
                            out.append({
                                "debug": ins.get("debug", 0),
                                "engine": ins["engine"],
                                "ins": [], "outs": [],
                                "name": f"WSPLIT-{seq[0]}",
                                "opcode": "NoOp",
                                "sync_info": {"on_update": [],
                                              "on_wait": extra[i:i + limit]},
                            })
                        si["on_wait"] = keep
                    out.append(ins)
                if changed:
                    bb["instructions"] = out
        return bir

    def _patched(bir_json, tmpdir, neff_name="file.neff", **kw):
        bir = split_bir_waits(json.loads(bir_json))
        return _orig(json.dumps(bir).encode(), tmpdir, neff_name, **kw)

    bass_utils.compile_bir_kernel = _patched
    bass_utils._wait_split_patched = True
    try:
        import concourse.bass2jax as bass2jax
        bass2jax.compile_bir_kernel = _patched
    except Exception:
        pass


def build_attention_nc():
    """Build the SPMD Bass program (one NeuronCore's view)."""
    import os
    # The default dependency tracker caps overlap-checking work per tensor
    # (max_work=100) and drops precision beyond it; the many small
    # subtile writes to the ct tiles here exceed that and the output
    # projection then races the normalize writes. Exhaustive checking
    # keeps every subtile dependency.
    os.environ["TILE_EXHAUSTIVE_MEMORY_SHARE_CHECK"] = "1"
    import concourse.bass as bass
    import concourse.mybir as mybir
    import concourse.tile as tile

    fp32 = mybir.dt.float32
    bf16 = mybir.dt.bfloat16
    Exp = mybir.ActivationFunctionType.Exp

    nc = bass.Bass("TRN2", target_bir_lowering=False, debug=False,
                   num_devices=N_CORES)

    xqT = nc.dram_tensor("xqT", [D, S], bf16, kind="ExternalInput")
    xkT = nc.dram_tensor("xkT", [D, S], bf16, kind="ExternalInput")
    xvT = nc.dram_tensor("xvT", [D, S], bf16, kind="ExternalInput")
    wq = nc.dram_tensor("wq", [D, GW], bf16, kind="ExternalInput")
    wk = nc.dram_tensor("wk", [D, GW], bf16, kind="ExternalInput")
    wv = nc.dram_tensor("wv", [D, GW], bf16, kind="ExternalInput")
    wo = nc.dram_tensor("wo", [GW, D], bf16, kind="ExternalInput")
    masks = nc.dram_tensor("masks", [128, 128], bf16, kind="ExternalInput")
    out = nc.dram_tensor("out", [S, D], fp32, kind="ExternalOutput")

    KC = D // 128    # 8 contraction chunks
    SKC = S // 128   # 16 key chunks

    with tile.TileContext(nc) as tc:
        with tc.tile_pool(name="wpool", bufs=1) as wpool, \
             tc.tile_pool(name="xpool", bufs=TUNE["xh"]) as xpool, \
             tc.tile_pool(name="persist", bufs=1) as persist, \
             tc.tile_pool(name="apool", bufs=TUNE["apool"]) as apool, \
             tc.tile_pool(name="rpool", bufs=2) as rpool, \
             tc.tile_pool(name="bpool", bufs=3) as bpool, \
             tc.tile_pool(name="tpool", bufs=2) as tpool, \
             tc.tile_pool(name="opool", bufs=4) as opool, \
             tc.tile_pool(name="pl", bufs=2, space="PSUM") as pl, \
             tc.tile_pool(name="pc", bufs=1, space="PSUM") as pc:

            wv_sb = wpool.tile([128, KC, GW], bf16, tag="wv")
            wk_sb = wpool.tile([128, KC, GW], bf16, tag="wk")
            wq_sb = wpool.tile([128, KC, GW], bf16, tag="wq")
            wo_sb = wpool.tile([128, GW // 128, D], bf16, tag="wo")
            mask_sb = wpool.tile([128, 128], bf16, tag="masks")

            qt = [persist.tile([128, S], bf16, tag=f"qt{m}", name=f"qt{m}")
                  for m in range(4)]
            kt = [persist.tile([128, S], bf16, tag=f"kt{m}", name=f"kt{m}")
                  for m in range(4)]
            vt = [persist.tile([128, HPG * (DK + 1)], bf16, tag=f"vt{j}",
                               name=f"vt{j}") for j in range(SKC)]
            ct = [persist.tile([128, S], bf16, tag=f"ct{m}", name=f"ct{m}")
                  for m in range(4)]

            # --- staged input loads: per-128-row chunk, per-1024-col half
            xh = {}   # (tensor_key, kc, half) -> sbuf tile [128, 1024]
            _dma_tog = [0]

            def dma_load(dst, src, pool=False):
                """Input loads go to the SP/HWDGE queue; pool=True routes
                via the Pool/SWDGE pipe instead — an independent issue path,
                used for bulk loads so they don't queue ahead of the
                latency-sensitive mid-kernel DMAs on HWDGE."""
                if pool and TUNE.get("pooldma", 0):
                    nc.gpsimd.dma_start(dst, src)
                else:
                    nc.sync.dma_start(dst, src)

            def load_w_half(w_sb, w_dram, h):
                dma_load(
                    w_sb[:, 4 * h:4 * (h + 1), :],
                    w_dram.ap().rearrange("(kc p) m -> p kc m", p=128)
                    [:, 4 * h:4 * (h + 1), :])

            def load_x_piece(key, xT, kc, h, p=0, split=1, pool=False):
                """Load piece p (of `split` column pieces) of x rows
                [128kc, 128kc+128) cols [1024h, 1024h+1024) into the
                (key, kc, h) tile. Pieces let compute start before the
                whole half has arrived (subtile deps gate per piece)."""
                t = xh.get((key, kc, h))
                if t is None:
                    t = xpool.tile([128, 1024], bf16, tag="xh", name="xh")
                    xh[(key, kc, h)] = t
                w = 1024 // split
                dma_load(
                    t[:, w * p:w * (p + 1)],
                    xT.ap()[128 * kc:128 * (kc + 1),
                            1024 * h + w * p:1024 * h + w * (p + 1)],
                    pool=pool)

            # ---- projection units (generators; ~2 matmuls per yield) ----
            def vproj_octo():
                """Prelude V proj for j0..7, kc-major across 8 live PSUM
                regions (2 psL tiles = 4 halves, 2 psP, and the 2 psC
                banks, idle until attention): each arriving x piece feeds
                8 matmuls (~1.7us of PE work per ~0.65us DMA), so the PE
                stays busy while the first loads dribble in."""
                psl = [pl.tile([128, 1024], fp32, tag="psL", name="psvq",
                               bufs=TUNE["psL"]) for _ in range(2)]
                ps = [psl[0][:, 0:512], psl[0][:, 512:1024],
                      psl[1][:, 0:512], psl[1][:, 512:1024],
                      pl.tile([128, 512], fp32, tag="psP", name="psvp",
                              bufs=TUNE["psP"]),
                      pl.tile([128, 512], fp32, tag="psP", name="psvp",
                              bufs=TUNE["psP"]),
                      pc.tile([128, 512], fp32, tag="pca", name="psvc"),
                      pc.tile([128, 512], fp32, tag="pcb", name="psvc")]
                for kc in range(KC):
                    for j in range(8):
                        nc.tensor.matmul(
                            ps[j], xh[("v", kc, 0)][:, 128 * j:128 * (j + 1)],
                            wv_sb[:, kc, :],
                            start=(kc == 0), stop=(kc == KC - 1))
                for j in range(8):
                    vt_v = vt[j][:].rearrange("p (h c) -> p h c", c=DK + 1)
                    nc.vector.tensor_copy(
                        vt_v[:, :, 0:DK],
                        ps[j].rearrange("p (h c) -> p h c", c=DK))
                    nc.vector.memset(vt_v[:, :, DK:DK + 1], 1.0)

            def vproj(j):
                """V proj for key chunk j -> vt[j] (natural + ones col)."""
                h = j // 8
                ps = pl.tile([128, 512], fp32, tag="psP", name="psv", bufs=TUNE["psP"])
                for kc in range(KC):
                    nc.tensor.matmul(
                        ps[:], xh[("v", kc, h)][:, 128 * (j % 8):128 * (j % 8 + 1)],
                        wv_sb[:, kc, :], start=(kc == 0), stop=(kc == KC - 1))
                    if kc == 3:
                        yield
                vt_v = vt[j][:].rearrange("p (h c) -> p h c", c=DK + 1)
                nc.vector.tensor_copy(
                    vt_v[:, :, 0:DK], ps[:].rearrange("p (h c) -> p h c", c=DK))
                nc.vector.memset(vt_v[:, :, DK:DK + 1], 1.0)
                yield

            def kqproj(which, m, js, act_copy=False, use_psl=False):
                """K or Q proj block -> kt/qt[m][:, 512js:512(js+1)].
                act_copy routes the PSUM->SBUF copy to ScalarE (idle during
                the prelude, when DVE is busy with V packing); use_psl takes
                PSUM from the psL tag (idle until attention starts) instead
                of contending with the V-quad's psP slots."""
                w_sb, dst, key = ((wk_sb, kt, "k") if which == "k"
                                  else (wq_sb, qt, "q"))
                h = js // 2
                if use_psl:
                    ps = pl.tile([128, 1024], fp32, tag="psL", name="pskql",
                                 bufs=TUNE["psL"])[:, 0:512]
                else:
                    ps = pl.tile([128, 512], fp32, tag="psP", name="pskq",
                                 bufs=TUNE["psP"])
                for kc in range(KC):
                    nc.tensor.matmul(
                        ps[:], w_sb[:, kc, 128 * m:128 * (m + 1)],
                        xh[(key, kc, h)][:, 512 * (js % 2):512 * (js % 2 + 1)],
                        start=(kc == 0), stop=(kc == KC - 1))
                    if kc == 3:
                        yield
                cpy = nc.scalar.copy if act_copy else nc.vector.tensor_copy
                cpy(dst[m][:, 512 * js:512 * (js + 1)], ps[:])
                yield

            # ---- attention unit (pr, h): psC in two 512-wide halves ------
            def normalize_half(pr, h, hb, ph):
                """Normalize one 512-wide psC half into ct: reciprocal of
                the denominator row, DMA row-broadcast, multiply. The DMA
                latency rides the DMA engines, off the in-order PE stream."""
                m, po = h // 2, (h % 2) * 64
                rc = rpool.tile([1, 512], fp32, tag="rc", name="rc")
                nc.vector.reciprocal(rc[:], ph[64:65, :])
                cs = ct[m][po:po + 64,
                           1024 * pr + 512 * hb:1024 * pr + 512 * (hb + 1)]
                cpy = (nc.scalar.copy if (TUNE.get("actcopy", 0) and pr == 0)
                       else nc.vector.tensor_copy)
                if po == 0:
                    cpy(cs, ph[0:64, :])
                else:
                    tmp = tpool.tile([64, 512], bf16, tag="tmp", name="tmp")
                    cpy(tmp[:], ph[0:64, :])
                    nc.sync.dma_start(cs, tmp[:])
                bc = bpool.tile([128, 512], fp32, tag="bc", name="bc")
                bch = bc[po:po + 64, :]
                nc.sync.dma_start(
                    bch, rc[0:1, :][:, None, :].to_broadcast((1, 64, 512)))
                nc.vector.tensor_mul(cs, cs, bch)

            def normalize_recip(ph):
                """Deferred variant, phase 1: reciprocal of the denominator
                row. bf16 is plenty for the 2e-2 tolerance."""
                rc = rpool.tile([128, 512], bf16, tag="rcd", name="rcd")
                with nc.allow_low_precision("bf16 softmax denominator recip; "
                                            "rel tolerance is 2e-2"):
                    nc.vector.reciprocal(rc[64:65, :], ph[64:65, :])
                return rc

            def normalize_finish(pr, h, hb, ph, rc):
                """Deferred variant, phase 2 (used for the final backbone
                unit where the tail is gated on this chain): broadcast
                recip across partitions 0..63 with a K=1 PE outer-product
                (mask_sb row 64, cols 64.. is all-ones) and normalize psC
                into ct in one fused multiply — ~1.8us instead of the
                ~4us DMA-broadcast chain."""
                m, po = h // 2, (h % 2) * 64
                bc = pl.tile([128, 512], fp32, tag="psP", name="bcp", bufs=TUNE["psP"])
                nc.tensor.matmul(bc[0:64, :], mask_sb[64:65, 64:128],
                                 rc[64:65, :], start=True, stop=True)
                bcs = bpool.tile([128, 512], fp32, tag="bc", name="bcs")
                nc.vector.tensor_copy(bcs[0:64, :], bc[0:64, :])
                cs = ct[m][po:po + 64,
                           1024 * pr + 512 * hb:1024 * pr + 512 * (hb + 1)]
                if po == 0:
                    nc.vector.tensor_mul(cs, ph[0:64, :], bcs[0:64, :])
                else:
                    tmp = tpool.tile([64, 512], bf16, tag="tmp", name="tmp")
                    nc.vector.tensor_mul(tmp[:], ph[0:64, :], bcs[0:64, :])
                    nc.sync.dma_start(cs, tmp[:])

            def attn_steps(pr, h, defer_finish=False, warm=False):
                """One head's attention. With warm=True the generator's
                first yield emits only QK0+exp0 (no psC write), so the
                driver can interleave it before the previous unit's final
                step: exp0 enqueues on ScalarE behind the old unit's last
                exps instead of after them, hiding the boundary latency.
                The first AV (the psC slot reuse) still comes after the
                previous unit's normalize emission, keeping WAR order."""
                m, po = h // 2, (h % 2) * 64
                qt_h = qt[m][po:po + 64, :]
                kt_h = kt[m][po:po + 64, :]
                nK = 8 * (pr + 1)
                pending = []

                def qk_exp(jk, off):
                    kt_sl = kt_h[:, 128 * jk:128 * (jk + 1)]
                    psL = pl.tile([128, 1024], fp32, tag="psL", name="psL", bufs=TUNE["psL"])
                    for lo, hi in ((off, 512), (max(off, 512), 1024)):
                        if lo >= hi:
                            continue
                        nc.tensor.matmul(
                            psL[:, lo:hi], kt_sl,
                            qt_h[:, 1024 * pr + lo:1024 * pr + hi],
                            start=True, stop=True)
                    at = apool.tile([128, 1024], bf16, tag="at", name="at")
                    nc.scalar.activation(at[:, off:1024], psL[:, off:1024],
                                         Exp, scale=0.125)
                    if 1024 * pr <= 128 * jk < 1024 * (pr + 1):
                        nc.vector.tensor_mul(at[:, off:off + 128],
                                             at[:, off:off + 128], mask_sb[:])
                    return at

                at0 = None
                if warm:
                    at0 = qk_exp(0, 0)
                    yield
                pa = pc.tile([128, 512], fp32, tag="pca", name="pa")[0:65, :]
                pb = pc.tile([128, 512], fp32, tag="pcb", name="pb")[0:65, :]
                halves = (pa, pb)
                for jk in range(nK):
                    for fin in pending:
                        fin()
                    pending.clear()
                    off = max(0, 128 * jk - 1024 * pr)
                    at = at0 if (warm and jk == 0) else qk_exp(jk, off)
                    vt_sl = vt[jk][:, (DK + 1) * h:(DK + 1) * (h + 1)]
                    for hb, lo, hi in ((0, off, 512), (1, max(off, 512), 1024)):
                        if lo >= hi:
                            continue
                        last = nK - 5 if hb == 0 else nK - 1
                        nc.tensor.matmul(
                            halves[hb][:, lo - 512 * hb:hi - 512 * hb],
                            vt_sl, at[:, lo:hi],
                            start=(jk == 0), stop=(jk == last))
                        if jk == last:
                            if defer_finish:
                                rc = normalize_recip(halves[hb])
                                pending.append(
                                    lambda hb=hb, ph=halves[hb], rc=rc:
                                    normalize_finish(pr, h, hb, ph, rc))
                            else:
                                normalize_half(pr, h, hb, halves[hb])
                    yield
                for fin in pending:
                    fin()

            # ---- output projection ---------------------------------------
            def oproj(pr, i_rel, width=512):
                i = 8 * pr + i_rel
                for n in range(D // width):
                    psO = pl.tile([128, 512], fp32, tag="psP",
                                  name="psO", bufs=TUNE["psP"])[:, 0:width]
                    for m in range(4):
                        nc.tensor.matmul(
                            psO[:], ct[m][:, 128 * i:128 * (i + 1)],
                            wo_sb[:, m, width * n:width * (n + 1)],
                            start=(m == 0), stop=(m == 3))
                    osb = opool.tile([128, 512], fp32, tag="osb",
                                     name="osb")[:, 0:width]
                    # tail oproj runs after the last exp: ScalarE is idle
                    # there while VectorE still drains normalize work —
                    # alternate the PSUM-freeing copies across both engines.
                    if pr == 1 and (2 * i_rel + n) % 2:
                        nc.scalar.copy(osb[:], psO[:])
                    else:
                        nc.vector.tensor_copy(osb[:], psO[:])
                    nc.sync.dma_start(
                        out.ap()[128 * i:128 * (i + 1),
                                 width * n:width * (n + 1)], osb[:])
                    yield

            # ---- DMA schedule (order = first use; each dma_start costs
            # ~625ns of serial HWDGE issue, so fewer+bigger wins) ----------
            wv_re = wv.ap().rearrange("(kc p) m -> p kc m", p=128)
            load_x_piece("v", xvT, 0, 0, 0, 2)
            dma_load(wv_sb[:, 0:1, :], wv_re[:, 0:1, :])
            load_x_piece("v", xvT, 0, 0, 1, 2)
            dma_load(wv_sb[:, 1:4, :], wv_re[:, 1:4, :])
            for kc in range(1, KC):
                load_x_piece("v", xvT, kc, 0)
                if kc == 1:
                    load_w_half(wv_sb, wv, 1)
                if kc == 2:
                    dma_load(mask_sb[:], masks.ap())
            load_w_half(wk_sb, wk, 0)
            for kc in range(KC):
                load_x_piece("k", xkT, kc, 0)
                if kc == 0:
                    load_w_half(wk_sb, wk, 1)
            load_w_half(wq_sb, wq, 0)
            for kc in range(KC):
                load_x_piece("q", xqT, kc, 0)
                if kc == 0:
                    load_w_half(wq_sb, wq, 1)
            for kc in range(KC):
                load_x_piece("v", xvT, kc, 1, pool=True)
            for kc in range(KC):
                load_x_piece("k", xkT, kc, 1, pool=True)
            for kc in range(KC):
                load_x_piece("q", xqT, kc, 1, pool=True)
            dma_load(wo_sb[:], wo.ap().rearrange("(m p) d -> p m d", p=128),
                     pool=True)

            def run(gen):
                for _ in gen:
                    pass

            # ---- prelude: enough projection work to start attention ------
            vproj_octo()
            run(kqproj("k", 0, 0, use_psl=True))
            run(kqproj("k", 0, 1, use_psl=True))
            run(kqproj("q", 0, 0, use_psl=True))
            run(kqproj("q", 0, 1, use_psl=True))

            # ---- backbone + fillers --------------------------------------
            # pr=1 ends on head 6 (po=0): an even head's normalize has no
            # partition-shift DMA in its chain, shortening the tail gate;
            # the final unit uses the deferred PE-outer-product normalize.
            # warm-start only the last units: there the filler queue is
            # empty, so pre-issuing the next unit's QK0+exp0 hides the
            # ScalarE boundary latency; earlier boundaries have filler and
            # the warm insert would only delay the previous unit's drain.
            NW = TUNE.get("warm_last", 4)
            wf = [False] * (16 - NW) + [True] * NW
            p1h = (0, 1, 2, 3, 4, 5, 7)
            backbone = [attn_steps(0, 0)] + \
                       [attn_steps(0, h, warm=wf[h]) for h in range(1, 8)] + \
                       [attn_steps(1, h, warm=wf[8 + i])
                        for i, h in enumerate(p1h)] + \
                       [attn_steps(1, 6, defer_finish=True, warm=wf[15])]
            nsteps = [8] * 8 + [16] * 8
            # Each filler is (min_step, deadline, gen). min_step delays
            # emission so filler is saved for the late backbone (where the
            # exp-vs-PE deficit would otherwise idle the PE); deadline is
            # the backbone step whose unit READS the filler's output — the
            # pull loop force-drains a filler as its deadline approaches,
            # because emission order is program order and a consumer emitted
            # before its producer reads garbage.
            # pr=1 unit (1,h) starts at step 65+16*idx; kt/qt js23 of group
            # m are first read by unit (1, 2m) (m=3: (1,7) at step 161).
            p1_start = {h: 65 + 16 * i
                        for i, h in enumerate((0, 1, 2, 3, 4, 5, 7, 6))}
            fillers = []
            for m in (1,):
                for js in (0, 1):
                    fillers.append((0, 16 * m + 1, kqproj("k", m, js)))
                    fillers.append((0, 16 * m + 1, kqproj("q", m, js)))
            for j in range(8, 16):
                fillers.append((0, p1_start[0], vproj(j)))
            for m in (2, 3):
                for js in (0, 1):
                    fillers.append((0, 16 * m + 1, kqproj("k", m, js)))
                    fillers.append((0, 16 * m + 1, kqproj("q", m, js)))
            for m in (0, 1, 2, 3):
                dl = min(p1_start[2 * m], p1_start[2 * m + 1])
                for js in (2, 3):
                    ms = max(0, TUNE["js23_base"] + TUNE["js23_step"] * m)
                    fillers.append((ms, dl, kqproj("k", m, js)))
                    fillers.append((ms, dl, kqproj("q", m, js)))
            # Emission order IS program order for the dependency tracker: a
            # filler that READS a tile must not be emitted before the
            # backbone step that WRITES it. min_step encodes that. oproj0
            # needs all pr=0 normalizes (emitted by step 64); oproj1 i8..11
            # need every pr=1 half-A normalize (last emitted at step 188);
            # i12..15 need the final half-B (step 192, i.e. post-loop).
            NODL = 10 ** 9
            for i_rel in range(8):
                fillers.append((TUNE["opj0_base"] + TUNE["opj0_step"] * i_rel,
                                NODL, oproj(0, i_rel)))
            for i_rel in range(4):
                fillers.append((189, NODL, oproj(1, i_rel)))
            for i_rel in range(4, 8):
                # 193 > last backbone step: these must not be emitted until
                # after the final unit's post-loop normalize finisher.
                fillers.append((193, NODL, oproj(1, i_rel)))

            step = [0]

            def pull(n):
                # force-drain fillers whose consumer unit is imminent
                i = 0
                while i < len(fillers):
                    min_step, dl, g = fillers[i]
                    if step[0] >= dl - 8:
                        for _ in g:
                            pass
                        del fillers[i]
                        continue
                    i += 1
                k = 0
                i = 0
                while i < len(fillers) and k < n:
                    min_step, dl, g = fillers[i]
                    if step[0] < min_step:
                        i += 1
                        continue
                    try:
                        next(g)
                        k += 1
                    except StopIteration:
                        del fillers[i]

            for ui, u in enumerate(backbone):
                for s in range(nsteps[ui]):
                    if (s == nsteps[ui] - 1 and ui + 1 < len(backbone)
                            and wf[ui + 1]):
                        next(backbone[ui + 1])  # next unit's warm QK0+exp0
                    next(u)
                    step[0] += 1
                    pull(TUNE["fill"])
                for _ in u:  # drain deferred normalize finishers
                    pass
            step[0] = 10 ** 9
            while fillers:
                pull(100)
    return nc


def make_masks():
    """Diagonal triangle mask [sk_r, sq_c]: keep (1.0) where c >= r."""
    r = np.arange(128)[:, None]
    c = np.arange(128)[None, :]
    return (c >= r).astype(BF16)


def make_in_maps(queries, keys, values, Wq, Wk, Wv, Wo):
    masks = make_masks()
    # per-batch transposed bf16 activations, shared by both head-group cores
    xT = {}
    for b in range(B):
        xT[b] = tuple(
            np.ascontiguousarray(np.asarray(x)[b].astype(BF16).T)
            for x in (queries, keys, values))
    wg = {}
    for g in range(2):
        sl = slice(g * GW, (g + 1) * GW)
        wg[g] = (np.asarray(Wq)[:, sl].astype(BF16),
                 np.asarray(Wk)[:, sl].astype(BF16),
                 np.asarray(Wv)[:, sl].astype(BF16),
                 np.ascontiguousarray(np.asarray(Wo)[sl, :]).astype(BF16))
    in_maps = []
    for c in range(N_CORES):
        b, g = c // 2, c % 2
        q, k, v = xT[b]
        wq_, wk_, wv_, wo_ = wg[g]
        in_maps.append({"xqT": q, "xkT": k, "xvT": v, "wq": wq_, "wk": wk_,
                        "wv": wv_, "wo": wo_, "masks": masks})
    return in_maps


def kernel(queries, keys, values, mask, Wq, Wk, Wv, Wo, bo):
    _apply_compat_patches()
    from concourse.bass_utils import run_bass_kernel_spmd

    key = "attn"
    if key not in _nc_cache:
        _nc_cache[key] = build_attention_nc()
    nc = _nc_cache[key]

    in_maps = make_in_maps(queries, keys, values, Wq, Wk, Wv, Wo)
    res = run_bass_kernel_spmd(nc, in_maps, core_ids=list(range(N_CORES)))

    out = np.empty((B, S, D), dtype=np.float32)
    for b in range(B):
        out[b] = res.results[2 * b]["out"] + res.results[2 * b + 1]["out"]
    out += bo.astype(np.float32)[None, None, :]
    return out
